# revision 1
# baseline (speedup 1.0000x reference)
"""DisplacementNet Trainium2 kernel: kNN graph + 4 GNN message-passing layers.

Sharding: data-parallel over nodes, 1024 rows per core (8 cores). Coordinates
and weights are replicated; the kNN candidate set is the full 8192 nodes. The
per-layer feature table is all-gathered across cores between layers.

Self-contained: hardcodes all shapes; no sibling imports.
"""
import numpy as np

import concourse.bass as bass
import concourse.bacc as bacc
import concourse.mybir as mybir
import concourse.tile as tile
from concourse import library_config
from concourse.tile_rust import add_dep_helper
from concourse.vector_clock import ScopedClock

N = 8192
NC = 8
NLOC = N // NC          # 1024
TPC = NLOC // 128       # 8 tiles of 128 nodes per core
K = 12
GW = 192
GL = 4
MIXW = 392              # 390 + ones + pad
LN_EPS = 1e-3
NEG_INF = -3.0e38
TABDT = "f16"           # exchanged-table dtype: "f16" | "f8"
EXCH = "full"           # "full" (column-split 8-core AllGathers) | "parity"
                        # (half AG + pair-shared table + barrier; needs
                        # reg-DMA on HW)
AG_SPLIT = 3            # column-split factor for EXCH=="full" (64 cols each);
                        # slices pinned to the DMA-capable engines so each
                        # slice's ib-write -> AllGather -> re-stride copy is
                        # ordered by the engine queue itself
F32 = mybir.dt.float32
AF = mybir.ActivationFunctionType
ALU = mybir.AluOpType
AX = mybir.AxisListType

# ---------------------------------------------------------------- tile patch

_MAXW = 1


def _patched_drain_and_barrier(self, tick_clock, wait_clock):
    nc = self.nc
    drain_inst = nc.sync.drain()
    wait_clock.add_sem_waits(
        drain_inst.ins, ScopedClock({None: tick_clock.global_clock})
    )
    si = drain_inst.ins.sync_info
    waits = list(si.on_wait or []) if si is not None else []
    if len(waits) > _MAXW:
        drain_inst.ins.sync_info = mybir.SyncInfo(
            on_wait=waits[:_MAXW], on_update=list(si.on_update or [])
        )
        rest = waits[_MAXW:]
        for i in range(0, len(rest), _MAXW):
            d2 = nc.sync.drain()
            d2.ins.sync_info = mybir.SyncInfo(on_wait=rest[i : i + _MAXW], on_update=[])
    nc.all_engine_barrier()
    assert self.sems is not None
    popped = nc._tile_sem_poison_stack.pop()
    assert popped is self._sem_poison
    nc.clear_and_free_semaphores(list(self.sems.allocated().values()))
    nc.all_engine_barrier()


tile.TileContext._drain_and_barrier = _patched_drain_and_barrier


def _split_waits(nc):
    """This walrus build allows one sync-wait per instruction; hoist extras
    onto same-engine NOPs inserted just before."""
    for f in nc.m.functions:
        for bb in f.blocks:
            out, changed = [], False
            for ins in bb.instructions:
                si = ins.sync_info
                waits = list(si.on_wait or []) if si is not None else []
                if len(waits) > 1:
                    changed = True
                    for i, w in enumerate(waits[:-1]):
                        nop = mybir.InstNoOp(
                            name=f"{ins.name}_wsplit{i}", engine=ins.engine,
                            ins=[], outs=[],
                        )
                        nop.sync_info = mybir.SyncInfo(on_wait=[w], on_update=[])
                        out.append(nop)
                    ins.sync_info = mybir.SyncInfo(
                        on_wait=[waits[-1]], on_update=list(si.on_update or [])
                    )
                out.append(ins)
            if changed:
                bb.instructions = out


# ---------------------------------------------------------------- kernel body

_GQ = [0]


def pool_on(eng, out_ap, in_ap3, func):
    """Emit InstPool (innermost-dim pooling) on an arbitrary engine — the
    bass helper only exists on the vector engine; the Pool engine is the
    natural home for it and is idle while the DVE runs the kNN chain.
    in_ap3 must be [p, X, W]; pooled over W. The unit-axis split makes the
    5d AP the ISA wants while staying symbolic-AP (TileContext) safe."""
    return eng.add_instruction(mybir.InstPool(
        name=f"I-{eng.bass.next_id()}", func=func,
        ins=[eng.lower_ap(in_ap3)], outs=[eng.lower_ap(out_ap)]))


def _fix_pools(nc):
    """Pad InstPool input APs to the 5d the ISA expects; must run after
    finalize because the tile AP canonicalizer merges unit/contiguous dims."""
    for f in nc.m.functions:
        for bb in f.blocks:
            for ins in bb.instructions:
                if isinstance(ins, mybir.InstPool):
                    ap = [list(d) for d in ins.ins[0].ap]
                    while len(ap) < 5:
                        ap.insert(1, [1, 1])
                    ins.ins[0].ap = mybir.VecI64Pair(ap)


def cc_on(eng, kind, op, groups, in_ap, out_ap):
    """Emit InstCollectiveCompute on an arbitrary engine queue (the bass
    helper only exists on gpsimd; the instruction itself is engine-agnostic)."""
    return eng.add_instruction(
        mybir.InstCollectiveCompute(
            name=f"I-{eng.bass.next_id()}",
            kind=kind, op=op, replica_groups=groups,
            ins=[eng.lower_ap(in_ap)], outs=[eng.lower_ap(out_ap)],
            unique_tensors="No", cc_dim="Partition"))


def gather_split(nc, out_tile, tab, widx_t, elem):
    """dma_gather with >1024 indices fails on HW; split into 1024 + 512.
    (Only SWDGE queue 0 exists on this build, so calls serialize there.)"""
    insts = []
    for (m0, cnt) in ((0, 1024), (1024, 512)):
        q = 0
        insts.append(nc.gpsimd.dma_gather(
            out_ap=out_tile[:, (m0 // 128) * elem:((m0 + cnt) // 128) * elem]
                .rearrange("p (j e) -> p j e", e=elem),
            in_ap=tab[:],
            idxs_ap=widx_t[:, m0 // 16:(m0 + cnt) // 16].bitcast(mybir.dt.int16),
            num_idxs=cnt, num_idxs_reg=cnt, elem_size=elem, queue_num=q))
    return insts


def build_kernel(n_cores=NC, split=True, phases="full"):
    nc = bacc.Bacc(None)
    nc.gpsimd.load_library(library_config.mlp)

    cons = nc.dram_tensor("cons", [52, N], F32, kind="ExternalInput")
    lhsD = nc.dram_tensor("lhsD", [4, NLOC], F32, kind="ExternalInput")
    Wp20 = nc.dram_tensor("Wp20", [20, GW], F32, kind="ExternalInput")
    WgA = nc.dram_tensor("WgA", [128, 16 * GW], F32, kind="ExternalInput")
    xsg = nc.dram_tensor("xsg", [128, 64 * 64], F32, kind="ExternalInput")
    xonn = nc.dram_tensor("xonn", [128, 3 * TPC], F32, kind="ExternalInput")
    feat_own = nc.dram_tensor("feat_own", [20, NLOC], F32, kind="ExternalInput")
    ident = nc.dram_tensor("ident", [128, 128], F32, kind="ExternalInput")
    Wo1 = nc.dram_tensor("Wo1", [128, 3], F32, kind="ExternalInput")
    Wo2 = nc.dram_tensor("Wo2", [65, 3], F32, kind="ExternalInput")
    pc = nc.dram_tensor("pc", [1, 4], mybir.dt.int32, kind="ExternalInput")
    y = nc.dram_tensor("y", [NLOC, 3], F32, kind="ExternalOutput")

    with tile.TileContext(nc) as tc:
        with tc.tile_pool(name="const", bufs=1) as cpool, \
             tc.tile_pool(name="big", bufs=1) as big, \
             tc.tile_pool(name="nd", bufs=2) as ndpool, \
             tc.tile_pool(name="gath", bufs=2) as gpool, \
             tc.tile_pool(name="small", bufs=2) as small, \
             tc.tile_pool(name="work", bufs=3) as work, \
             tc.tile_pool(name="tbp", bufs=2) as tbp, \
             tc.tile_pool(name="partp", bufs=8) as partp, \
             tc.tile_pool(name="g1p", bufs=1) as g1p, \
             tc.tile_pool(name="ndps", bufs=2, space="PSUM") as ndps, \
             tc.tile_pool(name="tps", bufs=2, space="PSUM") as tps, \
             tc.tile_pool(name="mmps", bufs=2, space="PSUM") as mmps, \
             tc.tile_pool(name="dram", bufs=1, space="DRAM") as dp:

            # ---- constants into SBUF (plain fp32: fp32r matmuls are lossy on
            # real silicon and flip near-tie kNN rankings)
            cons_sb = cpool.tile([52, N], F32)
            # column-split load so tile 0's dist matmuls + DVE chain start
            # ~8us earlier (cost model: per-partition free bytes per DMA)
            nc.sync.dma_start(out=cons_sb[:, 0:N // 2], in_=cons[:, 0:N // 2])
            nc.sync.dma_start(out=cons_sb[:, N // 2:N], in_=cons[:, N // 2:N])
            rhs_nd = cons_sb[0:4, :]
            feat_sb = cons_sb[32:52, :]
            lhs_sb = cpool.tile([4, NLOC], F32)
            nc.sync.dma_start(out=lhs_sb[:], in_=lhsD[:])
            Wp_sb = cpool.tile([52, GW], F32)
            nc.sync.dma_start(out=Wp_sb[32:52, :], in_=Wp20[:])
            Wg_sb = cpool.tile([128, 16 * GW], F32)
            nc.sync.dma_start(out=Wg_sb[:], in_=WgA[:])
            xonn_sb = cpool.tile([128, 3 * TPC], F32)
            nc.sync.dma_start(out=xonn_sb[:], in_=xonn[:])
            fown_sb = cpool.tile([20, NLOC], F32)
            nc.sync.dma_start(out=fown_sb[:], in_=feat_own[:])
            Wp0_sb = cpool.tile([20, GW], F32)
            nc.sync.dma_start(out=Wp0_sb[:], in_=Wp20[:])
            id_sb = cpool.tile([128, 128], F32)
            nc.sync.dma_start(out=id_sb[:], in_=ident[:])
            Wo1_sb = cpool.tile([128, 3], F32)
            nc.sync.dma_start(out=Wo1_sb[:], in_=Wo1[:])
            Wo2_sb = cpool.tile([65, 3], F32)
            nc.sync.dma_start(out=Wo2_sb[:], in_=Wo2[:])
            eps_sb = cpool.tile([128, 1], F32)
            nc.vector.memset(eps_sb[:], LN_EPS)

            # ---- DRAM buffers
            TDT = mybir.dt.float8e4 if TABDT == "f8" else mybir.dt.float16
            TE = 256
            tab0 = dp.tile([N, 256], F32)
            tabs = [tab0]
            for l in range(1, GL):
                kw = dict(addr_space="Shared") if EXCH == "parity" else {}
                tabs.append(dp.tile([N, TE], TDT, name=f"tab{l}", **kw))
            if EXCH == "parity":
                ibs = [dp.tile([NLOC, TE], TDT, name=f"ib{l}") for l in range(GL - 1)]
            else:
                # contiguous 192-col AllGather scratch (walrus requires
                # contiguous collective APs, and collectives may only run on
                # the Pool queue): trimmed payload (cols 192:256 never read)
                ibX = [dp.tile([NLOC, GW], TDT, name=f"ib{l}")
                       for l in range(GL - 1)]
                gX = [dp.tile([N, GW], TDT, name=f"gX{l}")
                      for l in range(GL - 1)]
            dwidxs = [dp.tile([16, 96], mybir.dt.uint16, name=f"dw{t}") for t in range(TPC)]
            if EXCH == "parity":
                # parity-exchange scratch: private AllGather landing buffer +
                # pair barrier in/out
                gbuf = dp.tile([N // 2, TE], TDT, name="gbuf")
                bar_in = dp.tile([1, 16], F32, name="bar_in")
                bar_out = dp.tile([2, 16], F32, name="bar_out")

                # parity (0 for even, 1 for odd) as an SP-engine register
                pc_sb = cpool.tile([1, 4], mybir.dt.int32)
                nc.sync.dma_start(out=pc_sb[:], in_=pc[:])
                parity = nc.values_load(pc_sb[0:1, 2:3],
                                        engines=[mybir.EngineType.SP],
                                        min_val=0, max_val=1)
                bi_sb = cpool.tile([1, 16], F32)
                nc.vector.memset(bi_sb[:], 0.0)
                nc.scalar.dma_start(out=bar_in[:], in_=bi_sb[:])

            # ---- persistent SBUF state
            widxs = [big.tile([128, 96], mybir.dt.uint16, name=f"wx{t}") for t in range(TPC)]
            mixbuf = big.tile([128, TPC * MIXW], F32)
            h4buf = big.tile([128, TPC * GW], F32)
            h16buf = big.tile([128, TPC * 256],
                              mybir.dt.float8e4 if TABDT == "f8" else mybir.dt.float16)
            nc.vector.memset(h16buf[:], 0.0)
            if EXCH == "full":
                # the column-split AllGathers only write cols 0:192; zero the
                # pad cols once so gathers never read uninitialized DRAM
                for l, eng in zip(range(1, GL), (nc.sync, nc.scalar, nc.gpsimd)):
                    for hh in range(2):
                        eng.dma_start(
                            out=tabs[l][hh * (N // 2):(hh + 1) * (N // 2), GW:TE]
                            .rearrange("(a p) e -> p a e", p=128),
                            in_=h16buf[:].rearrange("p (a e) -> p a e", e=TE - GW))
            ybuf = big.tile([128, TPC * 3], F32)
            mix3 = mixbuf[:].rearrange("p (t c) -> p t c", c=MIXW)
            nc.vector.memset(mix3[:, :, 390:391], 1.0)
            nc.vector.memset(mix3[:, :, 391:392], 0.0)

            # ---- phase A: full hcur0 table (graph_proj), every core redundantly
            for G in range(8):
                tb = tbp.tile([128, 8 * GW], F32, tag="tb")
                for i in range(8):
                    T = 8 * G + i
                    ps = mmps.tile([128, GW], F32, tag="mmps")
                    nc.tensor.matmul(ps[:], lhsT=feat_sb[:, 128 * T:128 * (T + 1)],
                                     rhs=Wp_sb[32:52, :], start=True, stop=True)
                    nc.scalar.copy(tb[:, GW * i:GW * (i + 1)], ps[:])
                # one batched DMA per 1024 rows: dst rows 128T+p
                dst = tab0[1024 * G:1024 * (G + 1), 0:GW].rearrange("(i p) e -> p i e", p=128)
                nc.scalar.dma_start(out=dst, in_=tb[:].rearrange("p (i e) -> p i e", e=GW))
                dstx = tab0[1024 * G:1024 * (G + 1), GW:256].rearrange("(i p) e -> p i e", p=128)
                nc.sync.dma_start(out=dstx, in_=xsg[:, 512 * G:512 * (G + 1)]
                                  .rearrange("p (i e) -> p i e", e=64))

            # own rows of hcur0 straight from per-core features (no gather)
            for t in range(TPC):
                ps = mmps.tile([128, GW], F32, tag="mmps")
                nc.tensor.matmul(ps[:], lhsT=fown_sb[:, 128 * t:128 * (t + 1)],
                                 rhs=Wp0_sb[:], start=True, stop=True)
                nc.scalar.copy(mix3[:, t, 0:GW], ps[:])

            if phases == "A":
                for t in range(TPC):
                    yt = small.tile([128, 3], F32, tag="yt")
                    nc.vector.tensor_copy(yt[:], mix3[:, t, 0:3])
                    nc.sync.dma_start(out=y[128 * t:128 * (t + 1), :], in_=yt[:])
            # ---- phase B: kNN + layer-1 gather per tile
            for t in range(TPC if phases != "A" else 0):
                negr = ndpool.tile([128, N], F32, tag="negr")
                for h in range(8):
                    ps = ndps.tile([128, 1024], F32, tag="ndps")
                    for q in range(2):
                        nc.tensor.matmul(
                            ps[:, 512 * q:512 * (q + 1)],
                            lhsT=lhs_sb[:, 128 * t:128 * (t + 1)],
                            rhs=rhs_nd[:, 1024 * h + 512 * q:1024 * h + 512 * (q + 1)],
                            start=True, stop=True)
                    nc.scalar.copy(negr[:, 1024 * h:1024 * (h + 1)], ps[:])
                # top-8 per 1024-col chunk: 64 candidates (losing a true
                # neighbor needs >=9 of the top-13 in one 1024-block — odds
                # ~2e-5 across all nodes)
                cand = small.tile([128, 64], F32, tag="cand")
                for i in range(8):
                    nc.vector.max(cand[:, 8 * i:8 * (i + 1)], negr[:, 1024 * i:1024 * (i + 1)])
                # top-16 of the 128 candidates; index searches write into one
                # contiguous tile so positions 1..12 DMA out directly
                v12 = small.tile([128, 16], F32, tag="v12")
                nc.vector.max(v12[:, 0:8], cand[:])
                cand2 = small.tile([128, 64], F32, tag="cand2")
                nc.vector.match_replace(cand2[:], v12[:, 0:8], cand[:], NEG_INF)
                nc.vector.max(v12[:, 8:16], cand2[:])
                i12 = small.tile([128, 16], mybir.dt.uint16, tag="i12")
                nc.vector.max_index(i12[:, 0:8], v12[:, 0:8], negr[:])
                nc.vector.max_index(i12[:, 8:16], v12[:, 8:16], negr[:])
                # wrapped idx build through DRAM: m = j*128+p at [m%16, 96t + m//16]
                # positions 1..12 of the global top-16 = the 12 nearest
                # neighbors (position 0 is self)
                d1 = dwidxs[t][:].rearrange("q (j phi) -> phi q j", phi=8)
                w1 = nc.sync.dma_start(out=d1, in_=i12[:, 1:13])
                # replicate the 16-partition wrapped list to all 8 groups:
                # 8 independent DRAM reads spread over 3 engines (the old
                # 16->32->64->128 doubling chain was 4 serial DMAs, ~9us of
                # latency on the phase-B exit path)
                reps = []
                repengs = [nc.sync, nc.scalar, nc.gpsimd]
                for k in range(8):
                    r = repengs[k % 3].dma_start(
                        out=widxs[t][16 * k:16 * (k + 1), :], in_=dwidxs[t][:])
                    add_dep_helper(r.ins, w1.ins, True, "rep after build")
                    reps.append(r)

                if phases == "BK":
                    yt = small.tile([128, 3], F32, tag="yt")
                    nc.vector.tensor_copy(yt[:], kn[:, 0:3])
                    nc.sync.dma_start(out=y[128 * t:128 * (t + 1), :], in_=yt[:])
                    continue
                # layer-1 feature gather; x/x2 stats ride along in cols 192:198
                # (SWDGE gather tops out at 1024 indices -> split 1024 + 512)
                g = g1p.tile([128, K * 256], F32, tag="g")
                gi = gather_split(nc, g, tab0, widxs[t], 256)
                if phases.startswith("BG2"):
                    yt = small.tile([128, 3], F32, tag="yt")
                    nc.vector.tensor_copy(yt[:], g[:, 0:3])
                    nc.sync.dma_start(out=y[128 * t:128 * (t + 1), :], in_=yt[:])
                    for r in reps:
                        for gg in gi:
                            add_dep_helper(gg.ins, r.ins, True, "gather after widx")
                    continue
                if phases == "BG":
                    gv = g[:].rearrange("p (j e) -> p j e", e=256)[:, :, 0:GW]
                    nc.vector.reduce_sum(mix3[:, t, GW:2 * GW],
                                   gv.rearrange("p j e -> p e j"), axis=AX.X)
                    yt = small.tile([128, 3], F32, tag="yt")
                    nc.vector.tensor_copy(yt[:], mix3[:, t, GW:GW + 3])
                    nc.sync.dma_start(out=y[128 * t:128 * (t + 1), :], in_=yt[:])
                    for r in reps:
                        for gg in gi:
                            add_dep_helper(gg.ins, r.ins, True, "gather after widx")
                    continue
                for r in reps:
                    for gg in gi:
                        add_dep_helper(gg.ins, r.ins, True, "gather after widx")

                # agg-sum for layer 1 (walrus: InstPool/TensorReduce-X are
                # DVE-only on Trn2, so this cannot offload to the Pool engine)
                gv = g[:].rearrange("p (j e) -> p j e", e=256)
                nc.vector.reduce_sum(mix3[:, t, GW:2 * GW],
                               gv[:, :, 0:GW].rearrange("p j e -> p e j"), axis=AX.X)
                # rel stats (layer-invariant): mean + population std via E[x2]-E[x]^2
                s6 = small.tile([128, 6], F32, tag="s6")
                nc.vector.reduce_sum(
                    s6[:], gv[:, :, GW:GW + 6].rearrange("p j c -> p c j"), axis=AX.X)
                m6 = small.tile([128, 6], F32, tag="m6")
                nc.vector.tensor_scalar_mul(m6[:], s6[:], 1.0 / K)
                nc.vector.tensor_sub(mix3[:, t, 384:387], m6[:, 0:3], xonn_sb[:, 3 * t:3 * (t + 1)])
                msq = small.tile([128, 3], F32, tag="msq")
                nc.vector.tensor_mul(msq[:], m6[:, 0:3], m6[:, 0:3])
                var3 = small.tile([128, 3], F32, tag="var3")
                nc.vector.tensor_sub(var3[:], m6[:, 3:6], msq[:])
                var3r = small.tile([128, 3], F32, tag="var3r")
                nc.vector.tensor_scalar_max(var3r[:], var3[:], 0.0)
                nc.scalar.activation(mix3[:, t, 387:390], var3r[:], AF.Sqrt)

            if phases == "B":
                for t in range(TPC):
                    yt = small.tile([128, 3], F32, tag="yt")
                    nc.vector.tensor_copy(yt[:], mix3[:, t, 384:387])
                    nc.sync.dma_start(out=y[128 * t:128 * (t + 1), :], in_=yt[:])
            # ---- phase C: GNN layers
            NLAYERS = GL if phases in ("full",) else (0 if phases.startswith(("A", "B")) else int(phases[1]))
            DO_AG = phases in ("full",) or phases.endswith("ag")
            def mm_chunk(t, l, j, pdst, start, stop):
                cj = 128 if j < 3 else 8
                pt = tps.tile([128, 128], F32, tag="tps", name="pt")
                nc.tensor.transpose(pt[0:cj, :], mix3[:, t, 128 * j:128 * j + cj], id_sb[:])
                lt = work.tile([128, 128], F32, tag="lt", name="lt")
                nc.scalar.copy(lt[0:cj, :], pt[0:cj, :])
                nc.tensor.matmul(pdst[:], lhsT=lt[0:cj, :],
                                 rhs=Wg_sb[0:cj, (4 * l + j) * GW:(4 * l + j + 1) * GW],
                                 start=start, stop=stop)

            parts = {}
            bars = {}
            for l in range(NLAYERS):
                for t in range(TPC):
                    if l > 0:
                        g = gpool.tile([128, K * TE], TDT, tag="g16")
                        gis = gather_split(nc, g, tabs[l], widxs[t], TE)
                        if l in bars:
                            for gg in gis:
                                add_dep_helper(gg.ins, bars[l].ins, True,
                                               "gather after pair barrier")
                        gv = g[:].rearrange("p (j e) -> p j e", e=TE)[:, :, 0:GW]
                        nc.vector.reduce_sum(mix3[:, t, GW:2 * GW],
                                       gv.rearrange("p j e -> p e j"), axis=AX.X)
                    if l == 0:
                        pmm = mmps.tile([128, GW], F32, tag="mmps")
                        for j in range(4):
                            mm_chunk(t, l, j, pmm, j == 0, j == 3)
                        zin = pmm
                    else:
                        # agg-dependent chunks only; chunks 0+3 were pre-run
                        # into parts[t] during the previous AllGather window
                        pmm = mmps.tile([128, GW], F32, tag="mmps")
                        mm_chunk(t, l, 1, pmm, True, False)
                        mm_chunk(t, l, 2, pmm, False, True)
                        zin = work.tile([128, GW], F32, tag="zadd", name="zadd")
                        nc.vector.tensor_add(zin[:], pmm[:], parts[t][:])
                    if l < GL - 1:
                        nc.scalar.activation(mix3[:, t, 0:GW], zin[:], AF.Silu)
                        nc.scalar.activation(h16buf[:, 256 * t:256 * t + GW], zin[:], AF.Silu)
                    else:
                        nc.scalar.activation(h4buf[:, GW * t:GW * (t + 1)], zin[:], AF.Silu)
                if l < GL - 1:
                    # exchange first so the Act-slice AllGather isn't queued
                    # behind the pre-pass copies on the Act engine
                    h3 = h16buf[:].rearrange("p (t e) -> p t e", e=256)
                    if EXCH == "parity":
                        dst = ibs[l][:].rearrange("(t p) e -> p t e", p=128)
                        nc.scalar.dma_start(out=dst, in_=h3)
                    else:
                        nc.sync.dma_start(
                            out=ibX[l][:].rearrange("(t p) e -> p t e", p=128),
                            in_=h3[:, :, 0:GW], max_dma_last_dim=GW)
                if l < GL - 1 and (DO_AG or l < NLAYERS - 1):
                    if EXCH == "parity":
                        # parity-group AllGather (half the payload), then place
                        # the 4096-row half into the pair-shared full table at a
                        # parity-dependent offset; pair barrier orders the
                        # partner half for the next layer's gathers.
                        nc.gpsimd.collective_compute(
                            "AllGather", ALU.bypass,
                            replica_groups=[[0, 2, 4, 6], [1, 3, 5, 7]],
                            ins=[ibs[l][:]], outs=[gbuf[:]])
                        cp = nc.sync.dma_start(
                            out=tabs[l + 1][bass.ts(parity, N // 2), :],
                            in_=gbuf[:], max_dma_last_dim=256)
                        bar = nc.gpsimd.collective_compute(
                            "AllGather", ALU.bypass,
                            replica_groups=[[0, 1], [2, 3], [4, 5], [6, 7]],
                            ins=[bar_in[:]], outs=[bar_out[:]])
                        add_dep_helper(bar.ins, cp.ins, True, "bar after table copy")
                        bars[l + 1] = bar
                    else:
                        # single Pool AllGather of the 192 data cols through
                        # contiguous scratch, then a cheap re-stride DMA into
                        # the 512B-row gather table (same queue: program order)
                        ag = nc.gpsimd.collective_compute(
                            "AllGather", ALU.bypass,
                            replica_groups=[list(range(n_cores))],
                            ins=[ibX[l][:]], outs=[gX[l][:]])
                        cp = nc.gpsimd.dma_start(out=tabs[l + 1][:, 0:GW],
                                                 in_=gX[l][:],
                                                 max_dma_last_dim=GW)
                        add_dep_helper(cp.ins, ag.ins, True,
                                       "restride after AG")
                    # pre-pass for next layer: agg-independent chunks, under
                    # the AllGather window (PE + Act work)
                    for t in range(TPC):
                        ph = mmps.tile([128, GW], F32, tag="mmps")
                        mm_chunk(t, l + 1, 0, ph, True, False)
                        mm_chunk(t, l + 1, 3, ph, False, True)
                        part = partp.tile([128, GW], F32, tag="part", name="part")
                        nc.vector.tensor_copy(part[:], ph[:])
                        parts[t] = part

            if phases.startswith("C"):
                for t in range(TPC):
                    yt = small.tile([128, 3], F32, tag="yt")
                    src = mix3[:, t, 0:3] if NLAYERS < GL else h4buf[:, GW * t:GW * t + 3]
                    nc.vector.tensor_copy(yt[:], src)
                    nc.sync.dma_start(out=y[128 * t:128 * (t + 1), :], in_=yt[:])
            # ---- phase D: LayerNorm (gamma/beta folded into Wo) + output proj
            for t in range(TPC if phases == "full" else 0):
                h4 = h4buf[:, GW * t:GW * (t + 1)]
                ssum = small.tile([128, 1], F32, tag="ssum")
                nc.vector.reduce_sum(ssum[:], h4, axis=AX.X)
                mu = small.tile([128, 1], F32, tag="mu")
                nc.vector.tensor_scalar_mul(mu[:], ssum[:], 1.0 / GW)
                xm = work.tile([128, GW], F32, tag="xm")
                nc.vector.tensor_scalar(xm[:], h4, mu[:], None, op0=ALU.subtract)
                sq = work.tile([128, GW], F32, tag="sq")
                vsum = small.tile([128, 1], F32, tag="vsum")
                nc.scalar.activation(sq[:], xm[:], AF.Square, accum_out=vsum[:])
                sd = small.tile([128, 1], F32, tag="sd")
                nc.scalar.activation(sd[:], vsum[:], AF.Sqrt, scale=1.0 / GW, bias=eps_sb[:])
                rin = small.tile([128, 1], F32, tag="rin")
                nc.vector.reciprocal(rin[:], sd[:])
                gn = work.tile([128, GW], F32, tag="gn")
                nc.vector.tensor_scalar_mul(gn[:], xm[:], rin[:])
                # transpose gn, then y = gn @ Wo' + bo'
                pz = mmps.tile([128, GW], F32, tag="mmps")
                pt1 = tps.tile([128, 128], F32, tag="tps")
                nc.tensor.transpose(pt1[:], gn[:, 0:128], id_sb[:])
                lt1 = work.tile([128, 128], F32, tag="lt")
                nc.scalar.copy(lt1[:], pt1[:])
                pt2 = tps.tile([128, 128], F32, tag="tps")
                nc.tensor.transpose(pt2[0:64, :], gn[:, 128:192], id_sb[:])
                lt2 = work.tile([128, 128], F32, tag="lt2")
                nc.scalar.copy(lt2[0:64, :], pt2[0:64, :])
                nc.vector.memset(lt2[64:65, :], 1.0)
                nc.tensor.matmul(pz[:, 0:3], lhsT=lt1[:], rhs=Wo1_sb[:], start=True, stop=False)
                nc.tensor.matmul(pz[:, 0:3], lhsT=lt2[0:65, :], rhs=Wo2_sb[:], start=False, stop=True)
                nc.scalar.copy(ybuf[:, 3 * t:3 * (t + 1)], pz[:, 0:3])
            if phases == "full":
                dst = y[:].rearrange("(t p) e -> p t e", p=128)
                nc.sync.dma_start(out=dst, in_=ybuf[:].rearrange("p (t e) -> p t e", e=3))

    nc.finalize()
    _fix_pools(nc)
    if split:
        _split_waits(nc)
    return nc


# ---------------------------------------------------------------- host side

def prep_inputs(x, z, B_fourier, Wp, bp, Wg, bg, gamma, beta, Wo, bo, n_cores=NC):
    x = np.asarray(x, np.float32); z = np.asarray(z, np.float32)
    B_fourier = np.asarray(B_fourier, np.float32)
    Wp = np.asarray(Wp, np.float32); bp = np.asarray(bp, np.float32)
    Wg = np.asarray(Wg, np.float32); bg = np.asarray(bg, np.float32)
    gamma = np.asarray(gamma, np.float32); beta = np.asarray(beta, np.float32)
    Wo = np.asarray(Wo, np.float32); bo = np.asarray(bo, np.float32)

    xb = x @ B_fourier                                  # (N, 8)
    featT = np.empty((20, N), np.float32)
    featT[0:8] = np.sin(xb).T
    featT[8:16] = np.cos(xb).T
    featT[16:19] = x.T
    featT[19] = 1.0
    x_sq = np.sum(x * x, axis=1)

    # table order tau: node n (core c = n>>10, offset m) lives at table row
    # (c&1)*4096 + (c>>1)*1024 + m — matches the parity AllGather layout.
    # (identity for the full-group exchange, whose table is in global order)
    nn = np.arange(N)
    if EXCH == "parity":
        cb, m = nn >> 10, nn & 1023
        tau = ((cb & 1) << 12) + ((cb >> 1) << 10) + m
    else:
        tau = nn
    inv = np.empty(N, np.int64)
    inv[tau] = nn                                       # new col r <- old col inv[r]

    cons = np.zeros((52, N), np.float32)
    cons[0:3] = x.T[:, inv]
    cons[3] = x_sq[inv]
    cons[32:52] = featT[:, inv]

    Wp20 = np.concatenate([Wp[0:19], (bp + z @ Wp[19:])[None]], 0).astype(np.float32)

    WgA = np.zeros((128, 16 * GW), np.float32)
    for l in range(GL):
        Wg_l = np.concatenate([
            Wg[l, 0:GW],
            Wg[l, GW:2 * GW] / K,
            Wg[l, 2 * GW:2 * GW + 6],
            bg[l][None],
            np.zeros((1, GW), np.float32),
        ], 0)                                            # (392, 192)
        for j in range(4):
            cj = 128 if j < 3 else 8
            WgA[0:cj, (4 * l + j) * GW:(4 * l + j + 1) * GW] = Wg_l[128 * j:128 * j + cj]

    xs_all = np.zeros((N, 64), np.float32)
    xs_all[:, 0:3] = x[inv]
    xs_all[:, 3:6] = (x * x)[inv]
    xsg = xs_all.reshape(64, 128, 64).transpose(1, 0, 2).reshape(128, 64 * 64)

    WoP = (gamma[:, None] * Wo).astype(np.float32)
    boP = (beta @ Wo + bo).astype(np.float32)
    Wo1 = WoP[0:128]
    Wo2 = np.concatenate([WoP[128:192], boP[None]], 0).astype(np.float32)

    ident = np.eye(128, dtype=np.float32)

    shared = {"cons": cons, "Wp20": Wp20, "WgA": WgA, "xsg": xsg,
              "ident": ident, "Wo1": Wo1, "Wo2": Wo2}
    in_maps = []
    for c in range(n_cores):
        rows = slice(NLOC * c, NLOC * (c + 1))
        xo = x[rows]                                     # (1024, 3)
        lhsD = np.empty((4, NLOC), np.float32)
        lhsD[0:3] = 2.0 * xo.T
        lhsD[3] = -1.0
        xonn = np.empty((128, 3 * TPC), np.float32)
        for t in range(TPC):
            xonn[:, 3 * t:3 * (t + 1)] = xo[128 * t:128 * (t + 1)]
        mm = dict(shared)
        pcv = np.array([[1 - (c & 1), c & 1, c & 1, 0]], np.int32)
        mm.update({"lhsD": lhsD, "xonn": xonn, "pc": pcv,
                   "feat_own": np.ascontiguousarray(featT[:, rows])})
        in_maps.append(mm)
    return in_maps


_CACHE = {}


def _get_nc(n_cores=NC):
    if n_cores not in _CACHE:
        _CACHE[n_cores] = build_kernel(n_cores)
    return _CACHE[n_cores]


def kernel(x, z, B_fourier, Wp, bp, Wg, bg, gamma, beta, Wo, bo):
    from concourse.bass_utils import run_bass_kernel_spmd
    nc = _get_nc(NC)
    in_maps = prep_inputs(x, z, B_fourier, Wp, bp, Wg, bg, gamma, beta, Wo, bo, NC)
    res = run_bass_kernel_spmd(nc, in_maps, list(range(NC)))
    return np.concatenate([res.results[c]["y"] for c in range(NC)], axis=0)



# revision 47
# speedup vs baseline: 1.0804x; 1.0804x over previous
"""DisplacementNet Trainium2 kernel: kNN graph + 4 GNN message-passing layers.

Sharding: data-parallel over nodes, 1024 rows per core (8 cores). Coordinates
and weights are replicated; the kNN candidate set is the full 8192 nodes. The
per-layer feature table is all-gathered across cores between layers (fp8
payload, 192 cols, gathered straight into the next layer's gather table).

kNN per 128-node tile: PE computes negated distances in 1024-col blocks; DVE
extracts per-block top-8 values+positions (one max8 + one max_index pass per
block), then selects the global top-13 among the 64 candidates and resolves
their node indices with a small SWDGE translate-gather.

Self-contained: hardcodes all shapes; no sibling imports.
"""
import numpy as np

import concourse.bass as bass
import concourse.bacc as bacc
import concourse.mybir as mybir
import concourse.tile as tile
from concourse import library_config
from concourse.tile_rust import add_dep_helper
from concourse.vector_clock import ScopedClock

N = 8192
NC = 8
NLOC = N // NC          # 1024
TPC = NLOC // 128       # 8 tiles of 128 nodes per core
K = 12
GW = 192
GL = 4
MIXW = 392              # 390 + ones + pad
T0W = 256               # tab0 row: 192 feat + 6 stats + pad (f32; gather rows
                        # must be 256B multiples)
TE = 256                # fp8 table row elems (256B gather granule)
LN_EPS = 1e-3
NEG_INF = -3.0e38
TABDT = "f16"            # exchanged-table dtype: "f8" | "f16"
F32 = mybir.dt.float32
BF16 = mybir.dt.bfloat16
U16 = mybir.dt.uint16
AF = mybir.ActivationFunctionType
ALU = mybir.AluOpType
AX = mybir.AxisListType

# ---------------------------------------------------------------- tile patch

_MAXW = 1


def _patched_drain_and_barrier(self, tick_clock, wait_clock):
    nc = self.nc
    drain_inst = nc.sync.drain()
    wait_clock.add_sem_waits(
        drain_inst.ins, ScopedClock({None: tick_clock.global_clock})
    )
    si = drain_inst.ins.sync_info
    waits = list(si.on_wait or []) if si is not None else []
    if len(waits) > _MAXW:
        drain_inst.ins.sync_info = mybir.SyncInfo(
            on_wait=waits[:_MAXW], on_update=list(si.on_update or [])
        )
        rest = waits[_MAXW:]
        for i in range(0, len(rest), _MAXW):
            d2 = nc.sync.drain()
            d2.ins.sync_info = mybir.SyncInfo(on_wait=rest[i : i + _MAXW], on_update=[])
    nc.all_engine_barrier()
    assert self.sems is not None
    popped = nc._tile_sem_poison_stack.pop()
    assert popped is self._sem_poison
    nc.clear_and_free_semaphores(list(self.sems.allocated().values()))
    nc.all_engine_barrier()


tile.TileContext._drain_and_barrier = _patched_drain_and_barrier


def _split_waits(nc):
    """This walrus build allows one sync-wait per instruction; hoist extras
    onto same-engine NOPs inserted just before."""
    for f in nc.m.functions:
        for bb in f.blocks:
            out, changed = [], False
            for ins in bb.instructions:
                si = ins.sync_info
                waits = list(si.on_wait or []) if si is not None else []
                if len(waits) > 1:
                    changed = True
                    for i, w in enumerate(waits[:-1]):
                        nop = mybir.InstNoOp(
                            name=f"{ins.name}_wsplit{i}", engine=ins.engine,
                            ins=[], outs=[],
                        )
                        nop.sync_info = mybir.SyncInfo(on_wait=[w], on_update=[])
                        out.append(nop)
                    ins.sync_info = mybir.SyncInfo(
                        on_wait=[waits[-1]], on_update=list(si.on_update or [])
                    )
                out.append(ins)
            if changed:
                bb.instructions = out


# ---------------------------------------------------------------- kernel body

def gather_idx(nc, out_tile, tab, widx_t, elem, splits=((0, 1024), (1024, 512))):
    """dma_gather with >1024 indices fails on HW; split into <=1024 chunks.
    (Only SWDGE queue 0 exists on this build, so calls serialize there.)"""
    insts = []
    for (m0, cnt) in splits:
        insts.append(nc.gpsimd.dma_gather(
            out_ap=out_tile[:, (m0 // 128) * elem:((m0 + cnt) // 128) * elem]
                .rearrange("p (j e) -> p j e", e=elem),
            in_ap=tab[:],
            idxs_ap=widx_t[:, m0 // 16:(m0 + cnt) // 16].bitcast(mybir.dt.int16),
            num_idxs=cnt, num_idxs_reg=cnt, elem_size=elem, queue_num=0))
    return insts


def build_kernel(n_cores=NC, split=True, phases="full"):
    nc = bacc.Bacc(None)
    nc.gpsimd.load_library(library_config.mlp)

    cons4 = nc.dram_tensor("cons4", [4, N], F32, kind="ExternalInput")
    featB = nc.dram_tensor("featB", [20, N], BF16, kind="ExternalInput")
    WpB = nc.dram_tensor("WpB", [20, GW], BF16, kind="ExternalInput")
    lhsD = nc.dram_tensor("lhsD", [4, NLOC], F32, kind="ExternalInput")
    Wp20 = nc.dram_tensor("Wp20", [20, GW], F32, kind="ExternalInput")
    WgA = nc.dram_tensor("WgA", [128, 16 * GW], F32, kind="ExternalInput")
    xsg = nc.dram_tensor("xsg", [128, 64 * 64], F32, kind="ExternalInput")
    xonn = nc.dram_tensor("xonn", [128, 3 * TPC], F32, kind="ExternalInput")
    feat_own = nc.dram_tensor("feat_own", [20, NLOC], F32, kind="ExternalInput")
    ident = nc.dram_tensor("ident", [128, 128], F32, kind="ExternalInput")
    Wo1 = nc.dram_tensor("Wo1", [128, 3], F32, kind="ExternalInput")
    Wo2 = nc.dram_tensor("Wo2", [65, 3], F32, kind="ExternalInput")
    base64D = nc.dram_tensor("base64", [128, 64], U16, kind="ExternalInput")
    pbaseD = nc.dram_tensor("pbase", [128, 12], U16, kind="ExternalInput")
    y = nc.dram_tensor("y", [NLOC, 3], F32, kind="ExternalOutput")

    with tile.TileContext(nc) as tc:
        with tc.tile_pool(name="const", bufs=1) as cpool, \
             tc.tile_pool(name="big", bufs=1) as big, \
             tc.tile_pool(name="nd", bufs=4) as ndpool, \
             tc.tile_pool(name="gath", bufs=2) as gpool, \
             tc.tile_pool(name="small", bufs=2) as small, \
             tc.tile_pool(name="work", bufs=3) as work, \
             tc.tile_pool(name="tbp", bufs=2) as tbp, \
             tc.tile_pool(name="fbp", bufs=1) as fbp, \
             tc.tile_pool(name="partp", bufs=8) as partp, \
             tc.tile_pool(name="g1p", bufs=2) as g1p, \
             tc.tile_pool(name="ndps", bufs=2, space="PSUM") as ndps, \
             tc.tile_pool(name="tps", bufs=2, space="PSUM") as tps, \
             tc.tile_pool(name="mmps", bufs=2, space="PSUM") as mmps, \
             tc.tile_pool(name="dram", bufs=1, space="DRAM") as dp:

            # ---- constants into SBUF (dist rows stay fp32: fp32r matmuls are
            # lossy on real silicon and flip near-tie kNN rankings)
            cons_sb = cpool.tile([4, N], F32)
            # 8-way split so tile 0's first dist matmul starts ~1us in
            for cc in range(8):
                nc.sync.dma_start(out=cons_sb[:, 1024 * cc:1024 * (cc + 1)],
                                  in_=cons4[:, 1024 * cc:1024 * (cc + 1)])
            rhs_nd = cons_sb[0:4, :]
            lhs_sb = cpool.tile([4, NLOC], F32)
            nc.sync.dma_start(out=lhs_sb[:], in_=lhsD[:])
            WpB_sb = cpool.tile([20, GW], BF16)
            nc.scalar.dma_start(out=WpB_sb[:], in_=WpB[:])
            Wg_sb = cpool.tile([128, 16 * GW], F32)
            nc.sync.dma_start(out=Wg_sb[:], in_=WgA[:])
            xonn_sb = cpool.tile([128, 3 * TPC], F32)
            nc.sync.dma_start(out=xonn_sb[:], in_=xonn[:])
            fown_sb = cpool.tile([20, NLOC], F32)
            nc.sync.dma_start(out=fown_sb[:], in_=feat_own[:])
            Wp0_sb = cpool.tile([20, GW], F32)
            nc.sync.dma_start(out=Wp0_sb[:], in_=Wp20[:])
            id_sb = cpool.tile([128, 128], F32)
            nc.sync.dma_start(out=id_sb[:], in_=ident[:])
            Wo1_sb = cpool.tile([128, 3], F32)
            nc.sync.dma_start(out=Wo1_sb[:], in_=Wo1[:])
            Wo2_sb = cpool.tile([65, 3], F32)
            nc.sync.dma_start(out=Wo2_sb[:], in_=Wo2[:])
            base64_sb = cpool.tile([128, 64], U16)
            nc.scalar.dma_start(out=base64_sb[:], in_=base64D[:])
            pbase_sb = cpool.tile([128, 12], U16)
            nc.scalar.dma_start(out=pbase_sb[:], in_=pbaseD[:])
            eps_sb = cpool.tile([128, 1], F32)
            nc.vector.memset(eps_sb[:], LN_EPS)

            TDT = mybir.dt.float8e4 if TABDT == "f8" else mybir.dt.float16
            # fp8 identity for PE pass-through accumulation (exact for 0/1)
            idq = cpool.tile([128, 128], TDT)
            nc.scalar.copy(idq[:], id_sb[:])

            # ---- DRAM buffers
            tab0 = dp.tile([N, T0W], F32)
            tabs = [tab0] + [dp.tile([N, TE], TDT, name=f"tab{l}")
                             for l in range(1, GL)]
            # contiguous AllGather scratch (walrus requires contiguous
            # collective APs; collectives may only run on the Pool queue)
            ibX = [dp.tile([NLOC, GW], TDT, name=f"ib{l}") for l in range(GL - 1)]
            gX = [dp.tile([N, GW], TDT, name=f"gX{l}") for l in range(GL - 1)]
            dwidxs = [dp.tile([16, 96], U16, name=f"dw{t}") for t in range(TPC)]
            dtw = [dp.tile([16, 96], U16, name=f"dt{t}") for t in range(TPC)]
            # candidate-index translate tables: row p*64+s is a 256B gather
            # granule whose first element holds gidx[p, s]
            gidxD = [dp.tile([N, 128], U16, name=f"gx{t}") for t in range(TPC)]

            # the AllGather restrides only write cols 0:GW; zero the pad cols
            # once so gathers never read uninitialized DRAM
            zpad = cpool.tile([128, 512], TDT)
            nc.vector.memset(zpad[:], 0.0)
            for l, eng in zip(range(1, GL), (nc.sync, nc.scalar, nc.gpsimd)):
                for hh in range(8):
                    eng.dma_start(
                        out=tabs[l][hh * 1024:(hh + 1) * 1024, GW:TE]
                        .rearrange("(a p) e -> p a e", p=128),
                        in_=zpad[:].rearrange("p (a e) -> p a e", e=TE - GW))

            # ---- persistent SBUF state
            widxs = [big.tile([128, 96], U16, name=f"wx{t}") for t in range(TPC)]
            mixbuf = big.tile([128, TPC * MIXW], F32)
            h4buf = big.tile([128, TPC * GW], F32)
            h8buf = big.tile([128, TPC * GW], TDT)
            ybuf = big.tile([128, TPC * 3], F32)
            mix3 = mixbuf[:].rearrange("p (t c) -> p t c", c=MIXW)
            nc.vector.memset(mix3[:, :, 390:391], 1.0)
            nc.vector.memset(mix3[:, :, 391:392], 0.0)

            # ---- phase A: full hcur0 table (graph_proj), every core redundantly
            # (bf16 features streamed in two halves to halve SBUF residency;
            # table writes spread over three DMA queues and PSUM drains
            # alternate Act/Pool so no single queue serializes A)
            wengs = [nc.scalar, nc.sync, nc.gpsimd]
            for HF in range(2):
                fb = fbp.tile([20, N // 2], BF16, tag="fb")
                nc.scalar.dma_start(out=fb[:], in_=featB[:, 4096 * HF:4096 * (HF + 1)])
                for G in range(8 * HF, 8 * (HF + 1)):
                    tb = tbp.tile([128, 4 * GW], F32, tag="tb")
                    for i in range(4):
                        T = 4 * G + i - 32 * HF
                        ps = mmps.tile([128, GW], F32, tag="mmps")
                        nc.tensor.matmul(ps[:], lhsT=fb[:, 128 * T:128 * (T + 1)],
                                         rhs=WpB_sb[:], start=True, stop=True)
                        # (GPSIMD cannot read PSUM on HW — Act drains all)
                        nc.scalar.copy(tb[:, GW * i:GW * (i + 1)], ps[:])
                    # one batched DMA per 512 rows: dst rows 128T+p
                    dst = tab0[512 * G:512 * (G + 1), 0:GW].rearrange("(i p) e -> p i e", p=128)
                    wengs[G % 3].dma_start(out=dst, in_=tb[:].rearrange("p (i e) -> p i e", e=GW))
            for G in range(8):
                dstx = tab0[1024 * G:1024 * (G + 1), GW:T0W].rearrange("(i p) e -> p i e", p=128)
                wengs[(G + 1) % 3].dma_start(out=dstx, in_=xsg[:, 512 * G:512 * (G + 1)]
                                             .rearrange("p (i e) -> p i e", e=64))

            # own rows of hcur0 straight from per-core features (no gather)
            for t in range(TPC):
                ps = mmps.tile([128, GW], F32, tag="mmps")
                nc.tensor.matmul(ps[:], lhsT=fown_sb[:, 128 * t:128 * (t + 1)],
                                 rhs=Wp0_sb[:], start=True, stop=True)
                nc.scalar.copy(mix3[:, t, 0:GW], ps[:])

            if phases == "A":
                for t in range(TPC):
                    yt = small.tile([128, 3], F32, tag="yt")
                    nc.vector.tensor_copy(yt[:], mix3[:, t, 0:3])
                    nc.sync.dma_start(out=y[128 * t:128 * (t + 1), :], in_=yt[:])

            # ---- phase B: kNN + layer-1 gather per tile, software-pipelined:
            # tile t's gather-dependent tail (L1 agg + rel stats) is emitted
            # during tile t+1 so neither the in-order PE queue nor the DVE
            # queue ever stalls waiting on the SWDGE gather
            repengs = [nc.sync, nc.scalar, nc.gpsimd]

            def b_tail(t, g1):
                # agg-sum for layer 1 on PE (12 fp32 identity pass-through
                # matmuls accumulating in PSUM) — the DVE is phase B's
                # bottleneck, so the 2.5us TensorReduce moves off it
                gv = g1[:].rearrange("p (j e) -> p j e", e=T0W)
                pa1 = mmps.tile([128, GW], F32, tag="mmps", name="pa1")
                for j in range(K):
                    nc.tensor.matmul(pa1[:], lhsT=id_sb[:],
                                     rhs=g1[:, T0W * j:T0W * j + GW],
                                     start=(j == 0), stop=(j == K - 1))
                nc.scalar.copy(mix3[:, t, GW:2 * GW], pa1[:])
                # rel stats (layer-invariant): mean + population std via E[x2]-E[x]^2
                s6 = small.tile([128, 6], F32, tag="s6")
                nc.vector.reduce_sum(
                    s6[:], gv[:, :, GW:GW + 6].rearrange("p j c -> p c j"), axis=AX.X)
                m6 = small.tile([128, 6], F32, tag="m6")
                nc.vector.tensor_scalar_mul(m6[:], s6[:], 1.0 / K)
                nc.vector.tensor_sub(mix3[:, t, 384:387], m6[:, 0:3], xonn_sb[:, 3 * t:3 * (t + 1)])
                msq = small.tile([128, 3], F32, tag="msq")
                nc.vector.tensor_mul(msq[:], m6[:, 0:3], m6[:, 0:3])
                var3 = small.tile([128, 3], F32, tag="var3")
                nc.vector.tensor_sub(var3[:], m6[:, 3:6], msq[:])
                var3r = small.tile([128, 3], F32, tag="var3r")
                nc.vector.tensor_scalar_max(var3r[:], var3[:], 0.0)
                nc.scalar.activation(mix3[:, t, 387:390], var3r[:], AF.Sqrt)

            pend = None
            for t in range(TPC if phases != "A" else 0):
                cand = small.tile([128, 64], F32, tag="cand")
                bidx = small.tile([128, 64], U16, tag="bidx")
                for h in range(8):
                    ps = ndps.tile([128, 1024], F32, tag="ndps")
                    for q in range(2):
                        nc.tensor.matmul(
                            ps[:, 512 * q:512 * (q + 1)],
                            lhsT=lhs_sb[:, 128 * t:128 * (t + 1)],
                            rhs=rhs_nd[:, 1024 * h + 512 * q:1024 * h + 512 * (q + 1)],
                            start=True, stop=True)
                    negr = ndpool.tile([128, 1024], F32, tag="negr")
                    nc.scalar.copy(negr[:], ps[:])
                    # per-block top-8 values + their in-block positions: one
                    # max8 pass + one max_index pass per 1024-col block (the
                    # old full-row max_index scanned 8192 cols twice)
                    nc.vector.max(cand[:, 8 * h:8 * (h + 1)], negr[:])
                    nc.vector.max_index(bidx[:, 8 * h:8 * (h + 1)], cand[:, 8 * h:8 * (h + 1)],
                                        negr[:])
                if pend is not None:
                    b_tail(*pend)
                # candidate global node index = in-block position + 1024*block
                gidx = small.tile([128, 64], U16, tag="gidx")
                nc.vector.tensor_add(gidx[:], bidx[:], base64_sb[:])
                w0 = nc.scalar.dma_start(
                    out=gidxD[t][:, 0:1].rearrange("(p s) e -> p s e", p=128),
                    in_=gidx[:].rearrange("p (s e) -> p s e", e=1))
                # top-16 of the 64 candidates (top-8 per 1024-block keeps the
                # true top-13 unless >=9 of them share a block: odds ~2e-5)
                v16 = small.tile([128, 16], F32, tag="v16")
                nc.vector.max(v16[:, 0:8], cand[:])
                cand2 = small.tile([128, 64], F32, tag="cand2")
                nc.vector.match_replace(cand2[:], v16[:, 0:8], cand[:], NEG_INF)
                nc.vector.max(v16[:, 8:16], cand2[:])
                pos16 = small.tile([128, 16], U16, tag="pos16")
                nc.vector.max_index(pos16[:, 0:8], v16[:, 0:8], cand[:])
                nc.vector.max_index(pos16[:, 8:16], v16[:, 8:16], cand[:])
                # positions 1..12 of the top-16 = the 12 nearest neighbors
                # (position 0 is self). Translate candidate slots -> node
                # indices with a tiny SWDGE gather from gidxD: table row for
                # (p, slot s) is p*64+s.
                tidx = small.tile([128, 12], U16, tag="tidx")
                nc.vector.tensor_add(tidx[:], pos16[:, 1:13], pbase_sb[:])
                wt = nc.sync.dma_start(
                    out=dtw[t][:].rearrange("q (j phi) -> phi q j", phi=8),
                    in_=tidx[:])
                ttw = work.tile([128, 96], U16, tag="ttw")
                reps1 = []
                for k in range(8):
                    r = repengs[k % 3].dma_start(
                        out=ttw[16 * k:16 * (k + 1), :], in_=dtw[t][:])
                    add_dep_helper(r.ins, wt.ins, True, "rep after tidx wrap")
                    reps1.append(r)
                t3 = small.tile([128, 12 * 128], U16, tag="t3")
                tg = gather_idx(nc, t3, gidxD[t], ttw, 128)
                for g in tg:
                    add_dep_helper(g.ins, w0.ins, True, "translate after gidx write")
                    for r in reps1:
                        add_dep_helper(g.ins, r.ins, True, "translate after rep")
                # wrapped final list -> DRAM -> replicate to all 8 SWDGE cores
                w1 = nc.sync.dma_start(
                    out=dwidxs[t][:].rearrange("q (j phi) -> phi q j", phi=8),
                    in_=t3[:].rearrange("p (j e) -> p j e", e=128)[:, :, 0:1])
                for g in tg:
                    add_dep_helper(w1.ins, g.ins, True, "wrap after translate")
                reps = []
                for k in range(8):
                    r = repengs[k % 3].dma_start(
                        out=widxs[t][16 * k:16 * (k + 1), :], in_=dwidxs[t][:])
                    add_dep_helper(r.ins, w1.ins, True, "rep after build")
                    reps.append(r)

                # layer-1 feature gather; x/x2 stats ride along in cols 192:198
                g1 = g1p.tile([128, K * T0W], F32, tag="g")
                gi = gather_idx(nc, g1, tab0, widxs[t], T0W)
                for r in reps:
                    for gg in gi:
                        add_dep_helper(gg.ins, r.ins, True, "gather after widx")
                pend = (t, g1)
            if pend is not None:
                b_tail(*pend)

            if phases == "B":
                for t in range(TPC):
                    yt = small.tile([128, 3], F32, tag="yt")
                    nc.vector.tensor_copy(yt[:], mix3[:, t, 384:387])
                    nc.sync.dma_start(out=y[128 * t:128 * (t + 1), :], in_=yt[:])

            # ---- phase C: GNN layers
            NLAYERS = GL if phases in ("full",) else (0 if phases.startswith(("A", "B")) else int(phases[1]))
            DO_AG = phases in ("full",) or phases.endswith("ag")

            def mm_chunk(t, l, j, pdst, start, stop):
                cj = 128 if j < 3 else 8
                pt = tps.tile([128, 128], F32, tag="tps", name="pt")
                nc.tensor.transpose(pt[0:cj, :], mix3[:, t, 128 * j:128 * j + cj], id_sb[:])
                lt = work.tile([128, 128], F32, tag="lt", name="lt")
                nc.scalar.copy(lt[0:cj, :], pt[0:cj, :])
                nc.tensor.matmul(pdst[:], lhsT=lt[0:cj, :],
                                 rhs=Wg_sb[0:cj, (4 * l + j) * GW:(4 * l + j + 1) * GW],
                                 start=start, stop=stop)

            def d_tile(t, mu8):
                # LayerNorm (gamma/beta folded into Wo) + output projection
                h4 = h4buf[:, GW * t:GW * (t + 1)]
                xm = work.tile([128, GW], F32, tag="xm")
                nc.vector.tensor_scalar(xm[:], h4, mu8[:, t:t + 1], None, op0=ALU.subtract)
                sq = work.tile([128, GW], F32, tag="sq")
                vsum = small.tile([128, 1], F32, tag="vsum")
                nc.scalar.activation(sq[:], xm[:], AF.Square, accum_out=vsum[:])
                sd = small.tile([128, 1], F32, tag="sd")
                nc.scalar.activation(sd[:], vsum[:], AF.Sqrt, scale=1.0 / GW, bias=eps_sb[:])
                rin = small.tile([128, 1], F32, tag="rin")
                nc.vector.reciprocal(rin[:], sd[:])
                gn = work.tile([128, GW], F32, tag="gn")
                nc.vector.tensor_scalar_mul(gn[:], xm[:], rin[:])
                # transpose gn, then y = gn @ Wo' + bo'
                pz = mmps.tile([128, GW], F32, tag="mmps")
                pt1 = tps.tile([128, 128], F32, tag="tps")
                nc.tensor.transpose(pt1[:], gn[:, 0:128], id_sb[:])
                lt1 = work.tile([128, 128], F32, tag="lt")
                nc.scalar.copy(lt1[:], pt1[:])
                pt2 = tps.tile([128, 128], F32, tag="tps")
                nc.tensor.transpose(pt2[0:64, :], gn[:, 128:192], id_sb[:])
                lt2 = work.tile([128, 128], F32, tag="lt2")
                nc.scalar.copy(lt2[0:64, :], pt2[0:64, :])
                nc.vector.memset(lt2[64:65, :], 1.0)
                nc.tensor.matmul(pz[:, 0:3], lhsT=lt1[:], rhs=Wo1_sb[:], start=True, stop=False)
                nc.tensor.matmul(pz[:, 0:3], lhsT=lt2[0:65, :], rhs=Wo2_sb[:], start=False, stop=True)
                nc.scalar.copy(ybuf[:, 3 * t:3 * (t + 1)], pz[:, 0:3])

            parts = {}
            ags = {}
            for l in range(NLAYERS):
                for t in range(TPC):
                    if l > 0:
                        g = gpool.tile([128, K * TE], TDT, tag="g8")
                        gis = gather_idx(nc, g, tabs[l], widxs[t], TE)
                        if l in ags:
                            for gg in gis:
                                for cp in ags[l]:
                                    add_dep_helper(gg.ins, cp.ins, True,
                                                   "gather after AG")
                        # neighbor-sum on PE: 12 pass-through matmuls with an
                        # fp8 identity accumulate the 12 gathered rows in PSUM
                        # (frees the DVE, which phase B saturates)
                        pa = mmps.tile([128, GW], F32, tag="mmps", name="pa")
                        for j in range(K):
                            nc.tensor.matmul(pa[:], lhsT=idq[:],
                                             rhs=g[:, TE * j:TE * j + GW],
                                             start=(j == 0), stop=(j == K - 1))
                        nc.scalar.copy(mix3[:, t, GW:2 * GW], pa[:])
                    if l == 0:
                        pmm = mmps.tile([128, GW], F32, tag="mmps")
                        for j in range(4):
                            mm_chunk(t, l, j, pmm, j == 0, j == 3)
                        zin = pmm
                    else:
                        # agg-dependent chunks only; chunks 0+3 were pre-run
                        # into parts[t] during the previous AllGather window
                        pmm = mmps.tile([128, GW], F32, tag="mmps")
                        mm_chunk(t, l, 1, pmm, True, False)
                        mm_chunk(t, l, 2, pmm, False, True)
                        zin = work.tile([128, GW], F32, tag="zadd", name="zadd")
                        nc.vector.tensor_add(zin[:], pmm[:], parts[t][:])
                    if l < GL - 1:
                        nc.scalar.activation(mix3[:, t, 0:GW], zin[:], AF.Silu)
                        nc.scalar.activation(h8buf[:, GW * t:GW * (t + 1)], zin[:], AF.Silu)
                    else:
                        nc.scalar.activation(h4buf[:, GW * t:GW * (t + 1)], zin[:], AF.Silu)
                if l < GL - 1:
                    # exchange first so the AllGather isn't queued behind the
                    # pre-pass copies on the Act engine
                    h3 = h8buf[:].rearrange("p (t e) -> p t e", e=GW)
                    nc.sync.dma_start(
                        out=ibX[l][:].rearrange("(t p) e -> p t e", p=128),
                        in_=h3, max_dma_last_dim=GW)
                if l < GL - 1 and (DO_AG or l < NLAYERS - 1):
                    # single Pool AllGather of the 192 fp8 cols through
                    # contiguous scratch, then a cheap re-stride copy into the
                    # 256B-row gather table (same queue: program order)
                    ag = nc.gpsimd.collective_compute(
                        "AllGather", ALU.bypass,
                        replica_groups=[list(range(n_cores))],
                        ins=[ibX[l][:]], outs=[gX[l][:]])
                    # re-stride split over the SP+Pool DMA queues (NOT Act:
                    # a cp waiting on the AG there would block the pre-pass
                    # copies behind the collective) so the next layer's
                    # gathers aren't queued behind one 9us copy
                    cp = nc.gpsimd.dma_start(out=tabs[l + 1][:, 0:GW],
                                             in_=gX[l][:],
                                             max_dma_last_dim=GW)
                    add_dep_helper(cp.ins, ag.ins, True, "restride after AG")
                    ags[l + 1] = [cp]
                    # pre-pass for next layer: agg-independent chunks, under
                    # the AllGather window (PE + Act work)
                    for t in range(TPC):
                        ph = mmps.tile([128, GW], F32, tag="mmps")
                        mm_chunk(t, l + 1, 0, ph, True, False)
                        mm_chunk(t, l + 1, 3, ph, False, True)
                        part = partp.tile([128, GW], F32, tag="part", name="part")
                        nc.vector.tensor_copy(part[:], ph[:])
                        parts[t] = part

            if phases.startswith("C"):
                for t in range(TPC):
                    yt = small.tile([128, 3], F32, tag="yt")
                    src = mix3[:, t, 0:3] if NLAYERS < GL else h4buf[:, GW * t:GW * t + 3]
                    nc.vector.tensor_copy(yt[:], src)
                    nc.sync.dma_start(out=y[128 * t:128 * (t + 1), :], in_=yt[:])

            if phases == "full":
                # batched per-row means for all 8 tiles in two DVE ops
                ssum8 = small.tile([128, TPC], F32, tag="ssum8")
                nc.vector.reduce_sum(
                    ssum8[:], h4buf[:].rearrange("p (t c) -> p t c", c=GW), axis=AX.X)
                mu8 = small.tile([128, TPC], F32, tag="mu8")
                nc.vector.tensor_scalar_mul(mu8[:], ssum8[:], 1.0 / GW)
                for t in range(TPC):
                    d_tile(t, mu8)
                dst = y[:].rearrange("(t p) e -> p t e", p=128)
                nc.sync.dma_start(out=dst, in_=ybuf[:].rearrange("p (t e) -> p t e", e=3))

    nc.finalize()
    if split:
        _split_waits(nc)
    return nc


# ---------------------------------------------------------------- host side

def prep_inputs(x, z, B_fourier, Wp, bp, Wg, bg, gamma, beta, Wo, bo, n_cores=NC):
    import ml_dtypes
    x = np.asarray(x, np.float32); z = np.asarray(z, np.float32)
    B_fourier = np.asarray(B_fourier, np.float32)
    Wp = np.asarray(Wp, np.float32); bp = np.asarray(bp, np.float32)
    Wg = np.asarray(Wg, np.float32); bg = np.asarray(bg, np.float32)
    gamma = np.asarray(gamma, np.float32); beta = np.asarray(beta, np.float32)
    Wo = np.asarray(Wo, np.float32); bo = np.asarray(bo, np.float32)

    xb = x @ B_fourier                                  # (N, 8)
    featT = np.empty((20, N), np.float32)
    featT[0:8] = np.sin(xb).T
    featT[8:16] = np.cos(xb).T
    featT[16:19] = x.T
    featT[19] = 1.0
    x_sq = np.sum(x * x, axis=1)

    cons4 = np.empty((4, N), np.float32)
    cons4[0:3] = x.T
    cons4[3] = x_sq

    Wp20 = np.concatenate([Wp[0:19], (bp + z @ Wp[19:])[None]], 0).astype(np.float32)
    featB = featT.astype(ml_dtypes.bfloat16)
    WpB = Wp20.astype(ml_dtypes.bfloat16)

    WgA = np.zeros((128, 16 * GW), np.float32)
    for l in range(GL):
        Wg_l = np.concatenate([
            Wg[l, 0:GW],
            Wg[l, GW:2 * GW] / K,
            Wg[l, 2 * GW:2 * GW + 6],
            bg[l][None],
            np.zeros((1, GW), np.float32),
        ], 0)                                            # (392, 192)
        for j in range(4):
            cj = 128 if j < 3 else 8
            WgA[0:cj, (4 * l + j) * GW:(4 * l + j + 1) * GW] = Wg_l[128 * j:128 * j + cj]

    xs_all = np.zeros((N, 64), np.float32)
    xs_all[:, 0:3] = x
    xs_all[:, 3:6] = x * x
    xsg = xs_all.reshape(64, 128, 64).transpose(1, 0, 2).reshape(128, 64 * 64)

    WoP = (gamma[:, None] * Wo).astype(np.float32)
    boP = (beta @ Wo + bo).astype(np.float32)
    Wo1 = WoP[0:128]
    Wo2 = np.concatenate([WoP[128:192], boP[None]], 0).astype(np.float32)

    ident = np.eye(128, dtype=np.float32)
    base64 = np.broadcast_to(
        ((np.arange(64) // 8) * 1024).astype(np.uint16)[None, :], (128, 64)).copy()
    pbase = np.broadcast_to(
        (np.arange(128, dtype=np.uint32) * 64).astype(np.uint16)[:, None],
        (128, 12)).copy()

    shared = {"cons4": cons4, "featB": featB, "WpB": WpB, "Wp20": Wp20,
              "WgA": WgA, "xsg": xsg, "ident": ident, "Wo1": Wo1, "Wo2": Wo2,
              "base64": base64, "pbase": pbase}
    in_maps = []
    for c in range(n_cores):
        rows = slice(NLOC * c, NLOC * (c + 1))
        xo = x[rows]                                     # (1024, 3)
        lhsD = np.empty((4, NLOC), np.float32)
        lhsD[0:3] = 2.0 * xo.T
        lhsD[3] = -1.0
        xonn = np.empty((128, 3 * TPC), np.float32)
        for t in range(TPC):
            xonn[:, 3 * t:3 * (t + 1)] = xo[128 * t:128 * (t + 1)]
        mm = dict(shared)
        mm.update({"lhsD": lhsD, "xonn": xonn,
                   "feat_own": np.ascontiguousarray(featT[:, rows])})
        in_maps.append(mm)
    return in_maps


_CACHE = {}


def _get_nc(n_cores=NC):
    if n_cores not in _CACHE:
        _CACHE[n_cores] = build_kernel(n_cores)
    return _CACHE[n_cores]


def kernel(x, z, B_fourier, Wp, bp, Wg, bg, gamma, beta, Wo, bo):
    from concourse.bass_utils import run_bass_kernel_spmd
    nc = _get_nc(NC)
    in_maps = prep_inputs(x, z, B_fourier, Wp, bp, Wg, bg, gamma, beta, Wo, bo, NC)
    res = run_bass_kernel_spmd(nc, in_maps, list(range(NC)))
    return np.concatenate([res.results[c]["y"] for c in range(NC)], axis=0)
